# revision 2
# baseline (speedup 1.0000x reference)
"""AudioTransformerMAE encoder (MoE routing) on 8 Trainium2 NeuronCores.

Strategy: pure data-parallel over batch (4 sequences per core); all weights
replicated; per-sequence top-4 expert weights fetched with indirect-gather
DMA in fp16 (halves the dominant HBM traffic; routing itself and everything
feeding it stays f32r/fp32 because the gate logits are nearly tied and flip
under bf16-level correlated noise).  Activations live feature-major
([D, token]) so every matmul uses weights-as-stored lhsT and tokens as the
moving dimension (N=500, full rate).  LayerNorm / softmax partition-axis
reductions go through the PE with a ones column; router runs in exact fp32.
"""
import os
import sys

for _p in ("/opt/trn_rl_repo", os.path.dirname(os.path.abspath(__file__))):
    if _p not in sys.path:
        sys.path.insert(0, _p)

import numpy as np

import concourse.bass as bass
import concourse.mybir as mybir
from concourse.tile import TileContext
from concourse.vector_clock import ScopedClock

f32 = mybir.dt.float32
f32r = mybir.dt.float32r
f16 = mybir.dt.float16
i32 = mybir.dt.int32
u32 = mybir.dt.uint32
AF = mybir.ActivationFunctionType
ALU = mybir.AluOpType
AX = mybir.AxisListType

# model dims
B, F, T = 32, 64, 2000
D, L, H, E, K = 768, 4, 12, 14, 4
HID = 2 * D
SHID = 4 * D
S = T // 4            # 500 tokens
PK = F * 4            # 256 patch input dim
NCORES = 8
BPC = B // NCORES     # 4 sequences per core
DC = D // 128         # 6 feature chunks
PC = PK // 128        # 2 patch chunks
HC = HID // 128       # 12
SC = SHID // 128      # 24
HPAIRS = H // 2       # 6 (two 64-dim heads per 128-col chunk)
EPS = 1e-5
ASCALE = (D // H) ** -0.5
# token chunks (partition tiles of the 500-token axis)
TCH = [(0, 128), (128, 128), (256, 128), (384, 116)]

MW1X = DC * HID + HC          # 9228: w1 k-chunks + b1 column block
MW2X = 3 * D + DC             # 2310: 3 w2 k-chunks + b2 column block

# debug-only reduced build (breaks correctness; build/compile testing only)
L_BUILD = int(os.environ.get("KM_L", L))
BPC_BUILD = int(os.environ.get("KM_BPC", BPC))


# ---------------------------------------------------------------- walrus fixups
def _patched_drain_and_barrier(self, tick_clock, wait_clock):
    nc = self.nc
    probe = nc.sync.nop(nofuse=True)
    wait_clock.add_sem_waits(probe.ins, ScopedClock({None: tick_clock.global_clock}))
    si = probe.ins.sync_info
    waits = list(si.on_wait) if si and si.on_wait else []
    if len(waits) > 1:
        si.on_wait = waits[:1]
        for w in waits[1:]:
            n2 = nc.sync.nop(nofuse=True)
            n2.ins.sync_info = mybir.SyncInfo(on_wait=[w], on_update=[])
    nc.sync.drain()
    nc.all_engine_barrier()
    popped = nc._tile_sem_poison_stack.pop()
    assert popped is self._sem_poison
    nc.clear_and_free_semaphores(list(self.sems.allocated().values()))
    nc.all_engine_barrier()


TileContext._drain_and_barrier = _patched_drain_and_barrier


def split_multi_waits(nc):
    """This walrus build encodes at most one sync wait per instruction; move
    extra waits onto same-engine NoOp spacers inserted just before."""
    n_split = 0
    for f in nc.m.functions:
        for bb in f.blocks:
            new_list = []
            for inst in bb.instructions:
                si = inst.sync_info
                waits = list(si.on_wait) if si and si.on_wait else []
                if len(waits) > 1:
                    for w in waits[:-1]:
                        nop = mybir.InstNoOp(
                            name=nc.get_next_instruction_name(),
                            engine=inst.engine,
                            ins=[],
                            outs=[],
                            sync_info=mybir.SyncInfo(on_wait=[w], on_update=[]),
                            bass_nofuse=True,
                        )
                        new_list.append(nop)
                        n_split += 1
                    si.on_wait = waits[-1:]
                new_list.append(inst)
            bb.instructions[:] = new_list
    return n_split


# ---------------------------------------------------------------- device kernel
def _col_ap(dram_1d):
    """[n*128] dram vector -> [128, n] (partition-major column layout)."""
    n = dram_1d.shape[0] // 128
    return dram_1d.rearrange("(c p) -> p c", p=128)


def _kxm_ap(dram_2d):
    """[Din, Dout] dram matrix -> [128, Din/128, Dout]."""
    return dram_2d.rearrange("(c p) j -> p c j", p=128)


def build_nc():
    nc = bass.Bass("TRN2")

    # ---- DRAM I/O (per-core shapes; weights replicated across cores) ----
    xT = nc.dram_tensor("xT", [BPC, PK, S], f32r, kind="ExternalInput")
    cw = nc.dram_tensor("cw", [PK, D], f32r, kind="ExternalInput")
    posc = nc.dram_tensor("posc", [D, S], f32, kind="ExternalInput")
    wqkv = nc.dram_tensor("wqkv", [L, D, 3 * D], f32r, kind="ExternalInput")
    wproj = nc.dram_tensor("wproj", [L, D, D], f32r, kind="ExternalInput")
    bproj = nc.dram_tensor("bproj", [L, D], f32, kind="ExternalInput")
    ln1s = nc.dram_tensor("ln1s", [L, D], f32, kind="ExternalInput")
    ln1b = nc.dram_tensor("ln1b", [L, D], f32, kind="ExternalInput")
    ln2s = nc.dram_tensor("ln2s", [L, D], f32, kind="ExternalInput")
    ln2b = nc.dram_tensor("ln2b", [L, D], f32, kind="ExternalInput")
    gw1 = nc.dram_tensor("gw1", [L, D, D], f32, kind="ExternalInput")
    gb1 = nc.dram_tensor("gb1", [L, D], f32, kind="ExternalInput")
    gw2 = nc.dram_tensor("gw2", [L, D, E], f32, kind="ExternalInput")
    gb2t = nc.dram_tensor("gb2t", [L, BPC, E], f32, kind="ExternalInput")
    # expert weights host-prepacked into per-partition row blocks so one
    # indirect DMA gathers a whole [128, X] tile (no register-offset DMAs):
    # mw1[l,e,p,:] = [w1[kc*128+p, :] for kc in 0..5] ++ b1col[p, 0:12]
    # mw2[l,e,q,p,:] = [w2[(3q+r)*128+p, :] for r in 0..2] ++ b2col[p, 0:6]
    mw1 = nc.dram_tensor("mw1", [L, E, 128, MW1X], f16, kind="ExternalInput")
    mw2 = nc.dram_tensor("mw2", [L, E, 4, 128, MW2X], f16,
                         kind="ExternalInput")
    indm = nc.dram_tensor("indm", [BPC, BPC * 128], f32r, kind="ExternalInput")
    iotf_d = nc.dram_tensor("iotf", [128, 1], f32, kind="ExternalInput")
    sw1 = nc.dram_tensor("sw1", [L, D, SHID], f32r, kind="ExternalInput")
    sb1 = nc.dram_tensor("sb1", [L, SHID], f32, kind="ExternalInput")
    sw2 = nc.dram_tensor("sw2", [L, SHID, D], f32r, kind="ExternalInput")
    sb2 = nc.dram_tensor("sb2", [L, D], f32, kind="ExternalInput")
    lnfs = nc.dram_tensor("lnfs", [D], f32, kind="ExternalInput")
    lnfb = nc.dram_tensor("lnfb", [D], f32, kind="ExternalInput")
    out_d = nc.dram_tensor("out", [BPC, D, S], f32, kind="ExternalOutput")

    mw1_view = mw1.rearrange("l e p x -> (l e p) x")
    mw2_view = mw2.rearrange("l e q p x -> (l e q p) x")

    from contextlib import ExitStack

    with TileContext(nc) as tc, ExitStack() as stack:
        # ---------------- persistent tiles ----------------
        pers = stack.enter_context(tc.tile_pool(name="pers", bufs=1))
        tok = [pers.tile([128, DC, S], f32r, tag=f"tok{s}", name=f"tok{s}")
               for s in range(BPC)]
        ones_f = pers.tile([128, 128], f32)       # fp32 ones (copy source)
        ones_col = pers.tile([128, 1], f32r)      # lhsT for partition sums
        ones_sq = pers.tile([128, 128], f32r)     # lhsT rows for broadcasts
        lnfs_c = pers.tile([128, DC], f32)
        lnfb_c = pers.tile([128, DC], f32)
        nc.vector.memset(ones_f[:], 1.0)
        nc.vector.tensor_copy(ones_col[:], ones_f[:, 0:1])
        nc.vector.tensor_copy(ones_sq[:], ones_f[:])
        ones_row = ones_sq[0:1, :]
        nc.sync.dma_start(lnfs_c[:], _col_ap(lnfs))
        nc.sync.dma_start(lnfb_c[:], _col_ap(lnfb))

        # ---------------- helpers ----------------
        def layernorm(src, dst, s_col, b_col, pool, psum_pool, sr_accum=None):
            """dst[:,kc,:] = LN(src) * s + b.  src/dst: [128, DC, S] (f32r).
            sr_accum: optional list of [128,1] APs per kc to accumulate
            sum-over-tokens of the OUTPUT (for the router mean)."""
            ps_sum = psum_pool.tile([1, S], f32, tag="ln_sum")
            ps_sq = psum_pool.tile([1, S], f32, tag="ln_sq")
            for kc in range(DC):
                nc.tensor.matmul(ps_sum[:], ones_col[:], src[:, kc, :],
                                 start=(kc == 0), stop=(kc == DC - 1))
            for kc in range(DC):
                sq = pool.tile([128, S], f32r, tag="ln_sqt", bufs=2)
                nc.vector.tensor_tensor(sq[:], src[:, kc, :].bitcast(f32),
                                        src[:, kc, :].bitcast(f32), op=ALU.mult)
                nc.tensor.matmul(ps_sq[:], ones_col[:], sq[:],
                                 start=(kc == 0), stop=(kc == DC - 1))
            m = pool.tile([1, S], f32, tag="ln_m")
            msq = pool.tile([1, S], f32, tag="ln_msq")
            var = pool.tile([1, S], f32, tag="ln_var")
            sd = pool.tile([1, S], f32, tag="ln_sd")
            rstd = pool.tile([1, S], f32, tag="ln_rstd")
            a_r = pool.tile([1, S], f32r, tag="ln_ar")
            nb_r = pool.tile([1, S], f32r, tag="ln_nbr")
            nc.vector.tensor_scalar(m[:], ps_sum[:], 1.0 / D, None, op0=ALU.mult)
            nc.vector.tensor_scalar(msq[:], ps_sq[:], 1.0 / D, None, op0=ALU.mult)
            nc.vector.tensor_tensor(var[:], m[:], m[:], op=ALU.mult)
            nc.vector.tensor_sub(var[:], msq[:], var[:])
            nc.vector.tensor_scalar(var[:], var[:], EPS, None, op0=ALU.add)
            nc.scalar.activation(sd[:], var[:], AF.Sqrt)
            nc.vector.reciprocal(rstd[:], sd[:])
            nc.vector.tensor_copy(a_r[:], rstd[:])
            nc.vector.tensor_tensor(nb_r[:], m[:], rstd[:], op=ALU.mult)
            nc.vector.tensor_scalar(nb_r[:], nb_r[:].bitcast(f32), -1.0, None,
                                    op0=ALU.mult)
            ps_a = psum_pool.tile([128, S], f32, tag="ln_bca")
            ps_b = psum_pool.tile([128, S], f32, tag="ln_bcb")
            nc.tensor.matmul(ps_a[:], ones_row[:], a_r[:], start=True, stop=True)
            nc.tensor.matmul(ps_b[:], ones_row[:], nb_r[:], start=True, stop=True)
            for kc in range(DC):
                t1 = pool.tile([128, S], f32, tag="ln_t1", bufs=2)
                nc.vector.tensor_tensor(t1[:], src[:, kc, :].bitcast(f32),
                                        ps_a[:], op=ALU.mult)
                nc.vector.tensor_tensor(t1[:], t1[:], ps_b[:], op=ALU.add)
                acc = sr_accum[kc] if sr_accum is not None else None
                nc.scalar.activation(dst[:, kc, :], t1[:], AF.Identity,
                                     bias=b_col[:, kc:kc + 1],
                                     scale=s_col[:, kc:kc + 1],
                                     accum_out=acc)

        # ---------------- patch embed ----------------
        with tc.tile_pool(name="patch", bufs=1) as pool, \
             tc.tile_pool(name="patch_ps", bufs=1, space="PSUM") as pp:
            cw_sb = pool.tile([128, PC, D], f32r)
            posc_sb = pool.tile([128, DC, S], f32)
            nc.sync.dma_start(cw_sb[:], _kxm_ap(cw))
            nc.sync.dma_start(posc_sb[:], _kxm_ap(posc))
            for s in range(BPC_BUILD):
                xt_sb = pool.tile([128, PC, S], f32r, tag="xt", bufs=2)
                nc.sync.dma_start(xt_sb[:], _kxm_ap(xT[s]))
                for mc in range(DC):
                    ps = pp.tile([128, S], f32, tag="patch", bufs=2)
                    for kc in range(PC):
                        nc.tensor.matmul(ps[:], cw_sb[:, kc, bass.ts(mc, 128)],
                                         xt_sb[:, kc, :],
                                         start=(kc == 0), stop=(kc == PC - 1))
                    nc.vector.tensor_tensor(tok[s][:, mc, :], ps[:],
                                            posc_sb[:, mc, :], op=ALU.add)

        # ---------------- layers ----------------
        for i in range(L_BUILD):
            # ======== attention ========
            with tc.tile_pool(name=f"attn{i}", bufs=1) as pool:
                wqkv_sb = pool.tile([128, DC, 3 * D], f32r)
                wproj_sb = pool.tile([128, DC, D], f32r)
                bproj_c = pool.tile([128, DC], f32)
                l1s_c = pool.tile([128, DC], f32)
                l1b_c = pool.tile([128, DC], f32)
                nc.sync.dma_start(wqkv_sb[:], _kxm_ap(wqkv[i]))
                nc.sync.dma_start(wproj_sb[:], _kxm_ap(wproj[i]))
                nc.sync.dma_start(bproj_c[:], _col_ap(bproj[i]))
                nc.sync.dma_start(l1s_c[:], _col_ap(ln1s[i]))
                nc.sync.dma_start(l1b_c[:], _col_ap(ln1b[i]))

                for s in range(BPC):
                    xn = pool.tile([128, DC, S], f32r, tag="xn_att", bufs=1)
                    with tc.tile_pool(name="ps_ln", bufs=1, space="PSUM") as pl:
                        layernorm(tok[s], xn, l1s_c, l1b_c, pool, pl)

                    # v in [token, feature] layout (+ ones col for softmax sum)
                    vml = pool.tile([128, 4, H, 65], f32r, tag="vml", bufs=1)
                    with tc.tile_pool(name="ps_v", bufs=1, space="PSUM") as pv:
                        for tc_i, (t0, tw) in enumerate(TCH):
                            for half in range(2):
                                psv = pv.tile([128, 384], f32, tag="v", bufs=2)
                                for kc in range(DC):
                                    nc.tensor.matmul(
                                        psv[0:tw, :],
                                        xn[:, kc, t0:t0 + tw],
                                        wqkv_sb[:, kc,
                                                2 * D + half * 384:
                                                2 * D + (half + 1) * 384],
                                        start=(kc == 0), stop=(kc == DC - 1))
                                nc.vector.tensor_copy(
                                    vml[0:tw, tc_i, 6 * half:6 * half + 6, 0:64],
                                    psv[0:tw, :].rearrange("p (h c) -> p h c", c=64))
                            nc.vector.tensor_copy(
                                vml[0:tw, tc_i, :, 64],
                                ones_f[0:tw, 0:1].to_broadcast([tw, H]))

                    attn_sb = pool.tile([128, DC, S], f32r, tag="attn_sb", bufs=1)
                    with tc.tile_pool(name="ps_hd", bufs=1, space="PSUM") as ph:
                        for hp in range(HPAIRS):
                            qk = pool.tile([128, 2, S], f32r, tag="qk", bufs=2)
                            for j, base in ((0, hp * 128), (1, D + hp * 128)):
                                psqk = ph.tile([128, S], f32, tag="qk", bufs=2)
                                for kc in range(DC):
                                    nc.tensor.matmul(
                                        psqk[:], wqkv_sb[:, kc, base:base + 128],
                                        xn[:, kc, :],
                                        start=(kc == 0), stop=(kc == DC - 1))
                                nc.vector.tensor_copy(qk[:, j, :], psqk[:])
                            for e in range(2):
                                h = 2 * hp + e
                                pb = e * 64
                                pso = ph.tile([65, S], f32, tag="o", bufs=1)
                                for tc_i, (t0, tw) in enumerate(TCH):
                                    pss = ph.tile([128, S], f32, tag="sc", bufs=2)
                                    nc.tensor.matmul(
                                        pss[0:tw, :],
                                        qk[pb:pb + 64, 1, t0:t0 + tw],
                                        qk[pb:pb + 64, 0, :],
                                        start=True, stop=True)
                                    ex = pool.tile([128, S], f32r, tag="exp", bufs=3)
                                    nc.scalar.activation(ex[0:tw, :], pss[0:tw, :],
                                                         AF.Exp, scale=ASCALE)
                                    nc.tensor.matmul(pso[:],
                                                     vml[0:tw, tc_i, h, :],
                                                     ex[0:tw, :],
                                                     start=(tc_i == 0),
                                                     stop=(tc_i == 3))
                                rr = pool.tile([1, S], f32r, tag="rr", bufs=2)
                                with nc.allow_low_precision(
                                        reason="f32r rounding of softmax denom"):
                                    nc.vector.reciprocal(rr[:], pso[64:65, :])
                                psb = ph.tile([64, S], f32, tag="hb", bufs=1)
                                nc.tensor.matmul(psb[:], ones_row[0:1, 0:64],
                                                 rr[:], start=True, stop=True)
                                hb = pool.tile([64, S], f32, tag="hb_sb", bufs=2)
                                nc.vector.tensor_copy(hb[:], psb[:])
                                nc.vector.tensor_tensor(
                                    attn_sb[pb:pb + 64, hp, :],
                                    pso[0:64, :], hb[:], op=ALU.mult)

                    # output projection + residual
                    with tc.tile_pool(name="ps_pr", bufs=1, space="PSUM") as pr:
                        for mc in range(DC):
                            psp = pr.tile([128, S], f32, tag="proj", bufs=2)
                            for kc in range(DC):
                                nc.tensor.matmul(
                                    psp[:], wproj_sb[:, kc, bass.ts(mc, 128)],
                                    attn_sb[:, kc, :],
                                    start=(kc == 0), stop=(kc == DC - 1))
                            nc.vector.scalar_tensor_tensor(
                                tok[s][:, mc, :], psp[:],
                                bproj_c[:, mc:mc + 1],
                                tok[s][:, mc, :].bitcast(f32),
                                op0=ALU.add, op1=ALU.add)

            # ======== MoE ========
            with tc.tile_pool(name=f"moe{i}", bufs=1) as pool:
                l2s_c = pool.tile([128, DC], f32)
                l2b_c = pool.tile([128, DC], f32)
                nc.sync.dma_start(l2s_c[:], _col_ap(ln2s[i]))
                nc.sync.dma_start(l2b_c[:], _col_ap(ln2b[i]))
                xnm = [pool.tile([128, DC, S], f32r, tag=f"xnm{s}", name=f"xnm{s}")
                       for s in range(BPC)]
                sr = pool.tile([128, DC, BPC], f32)   # sum over tokens of xn
                wb_all = pool.tile([128, BPC * K], f32)
                # routed-expert b2 contribution, weighted: sum_k wb_k * b2_k
                b2acc = [pool.tile([128, DC], f32, tag=f"b2a{s}", name=f"b2a{s}")
                         for s in range(BPC)]
                # per-pair gather offsets: col j*5 = mw1 row, j*5+1+q = mw2 rows
                off_all = pool.tile([128, BPC * K * 5], i32)

                for s in range(BPC):
                    with tc.tile_pool(name="ps_ln2", bufs=1, space="PSUM") as pl:
                        layernorm(tok[s], xnm[s], l2s_c, l2b_c, pool, pl,
                                  sr_accum=[sr[:, kc, s:s + 1] for kc in range(DC)])

                # ---- router (exact fp32) ----
                with tc.tile_pool(name="gate", bufs=1) as gp, \
                     tc.tile_pool(name="ps_g", bufs=1, space="PSUM") as pg:
                    gw1_sb = gp.tile([128, DC, D], f32)
                    gw2_sb = gp.tile([128, DC, E], f32)
                    gb1_c = gp.tile([128, DC], f32)
                    gb2_sb = gp.tile([BPC, E], f32)
                    nc.sync.dma_start(gw1_sb[:], _kxm_ap(gw1[i]))
                    nc.sync.dma_start(gw2_sb[:], _kxm_ap(gw2[i]))
                    nc.sync.dma_start(gb1_c[:], _col_ap(gb1[i]))
                    nc.sync.dma_start(gb2_sb[:], gb2t[i])
                    g1 = gp.tile([128, DC, BPC], f32)
                    for mc in range(DC):
                        psg = pg.tile([128, BPC], f32, tag="g1", bufs=2)
                        for kc in range(DC):
                            nc.tensor.matmul(psg[:],
                                             gw1_sb[:, kc, bass.ts(mc, 128)],
                                             sr[:, kc, :],
                                             start=(kc == 0), stop=(kc == DC - 1))
                        nc.scalar.activation(g1[:, mc, :], psg[:], AF.Gelu,
                                             bias=gb1_c[:, mc:mc + 1],
                                             scale=1.0 / S)
                    psl = pg.tile([BPC, E], f32, tag="logits")
                    for kc in range(DC):
                        nc.tensor.matmul(psl[:], g1[:, kc, :], gw2_sb[:, kc, :],
                                         start=(kc == 0), stop=(kc == DC - 1))
                    logits = gp.tile([BPC, E], f32)
                    nc.vector.tensor_tensor(logits[:], psl[:], gb2_sb[:], op=ALU.add)
                    # logits are O(0.01): exp without max-subtraction is safe
                    pe_t = gp.tile([BPC, E], f32)
                    esum = gp.tile([BPC, 1], f32)
                    nc.scalar.activation(pe_t[:], logits[:], AF.Exp,
                                         accum_out=esum[:])
                    erec = gp.tile([BPC, 1], f32)
                    nc.vector.reciprocal(erec[:], esum[:])
                    probs = gp.tile([BPC, E], f32)
                    nc.vector.tensor_scalar(probs[:], pe_t[:], erec[:, 0:1], None,
                                            op0=ALU.mult)
                    m8 = gp.tile([BPC, 8], f32)
                    i8 = gp.tile([BPC, 8], u32)
                    nc.vector.max_with_indices(m8[:], i8[:], probs[:])
                    idxf_r = gp.tile([BPC, K], f32r)
                    nc.vector.tensor_copy(idxf_r[:], i8[:, 0:K])
                    wex = gp.tile([BPC, K], f32)
                    wsum = gp.tile([BPC, 1], f32)
                    nc.scalar.activation(wex[:], m8[:, 0:K], AF.Exp,
                                         accum_out=wsum[:])
                    wrec = gp.tile([BPC, 1], f32)
                    nc.vector.reciprocal(wrec[:], wsum[:])
                    wn_r = gp.tile([BPC, K], f32r)
                    nc.vector.tensor_scalar(wn_r[:], wex[:], wrec[:, 0:1], None,
                                            op0=ALU.mult)
                    # indm[s, s*128:(s+1)*128] = 1 (host const): one N=K matmul
                    # broadcasts row s of wn / idx across 128 partitions
                    ind_r = gp.tile([BPC, BPC * 128], f32r)
                    iotf = gp.tile([128, 1], f32)
                    nc.sync.dma_start(ind_r[:], indm[:])
                    nc.sync.dma_start(iotf[:], iotf_d[:])
                    # per-layer row-base columns for the two mega views
                    iot1 = gp.tile([128, 1], f32)
                    iot2 = [gp.tile([128, 1], f32, tag=f"iot2_{q}",
                                    name=f"iot2_{q}") for q in range(4)]
                    nc.vector.tensor_scalar(iot1[:], iotf[:], float(i * E * 128),
                                            None, op0=ALU.add)
                    for q in range(4):
                        nc.vector.tensor_scalar(
                            iot2[q][:], iotf[:],
                            float(i * E * 4 * 128 + q * 128), None, op0=ALU.add)
                    with tc.tile_pool(name="ps_wb", bufs=1, space="PSUM") as pw:
                        for s in range(BPC):
                            psw = pw.tile([128, K], f32, tag="wb", bufs=2)
                            nc.tensor.matmul(psw[:],
                                             ind_r[0:BPC, bass.ts(s, 128)],
                                             wn_r[0:BPC, 0:K],
                                             start=True, stop=True)
                            nc.vector.tensor_copy(
                                wb_all[:, s * K:(s + 1) * K], psw[:])
                            psi = pw.tile([128, K], f32, tag="ib", bufs=2)
                            nc.tensor.matmul(psi[:],
                                             ind_r[0:BPC, bass.ts(s, 128)],
                                             idxf_r[0:BPC, 0:K],
                                             start=True, stop=True)
                            for k in range(K):
                                j = s * K + k
                                of1 = gp.tile([128, 1], f32, tag="of1", bufs=2)
                                nc.vector.tensor_scalar(
                                    of1[:], psi[:, k:k + 1], 128.0, iot1[:, 0:1],
                                    op0=ALU.mult, op1=ALU.add)
                                nc.vector.tensor_copy(
                                    off_all[:, j * 5:j * 5 + 1], of1[:])
                                for q in range(4):
                                    of2 = gp.tile([128, 1], f32, tag="of2",
                                                  bufs=2)
                                    nc.vector.tensor_scalar(
                                        of2[:], psi[:, k:k + 1], 512.0,
                                        iot2[q][:, 0:1],
                                        op0=ALU.mult, op1=ALU.add)
                                    nc.vector.tensor_copy(
                                        off_all[:, j * 5 + 1 + q:j * 5 + 2 + q],
                                        of2[:])

                # ---- routed experts (indirect weight gather) ----
                with tc.tile_pool(name="exps", bufs=1) as xp, \
                     tc.tile_pool(name="ps_e", bufs=1, space="PSUM") as pe:
                    for s in range(BPC):
                        # fp16 copy of the normed input for the expert matmuls
                        xnm16 = xp.tile([128, DC, S], f16, tag=f"x16_{s}",
                                        name=f"x16_{s}")
                        for kc in range(DC):
                            nc.vector.tensor_copy(
                                xnm16[:, kc, :],
                                xnm[s][:, kc, :].bitcast(f32))
                        for k in range(K):
                            j = s * K + k
                            w1m = xp.tile([128, MW1X], f16, tag="w1", bufs=2)
                            nc.gpsimd.indirect_dma_start(
                                out=w1m[:], out_offset=None, in_=mw1_view,
                                in_offset=bass.IndirectOffsetOnAxis(
                                    ap=off_all[:, j * 5:j * 5 + 1], axis=0))
                            h1ch = []
                            for hc in range(HC):
                                psh = pe.tile([128, S], f32, tag="h1", bufs=2)
                                for kc in range(DC):
                                    nc.tensor.matmul(
                                        psh[:],
                                        w1m[:, kc * HID + hc * 128:
                                            kc * HID + (hc + 1) * 128],
                                        xnm16[:, kc, :],
                                        start=(kc == 0), stop=(kc == DC - 1))
                                hg = xp.tile([128, S], f16, tag="h1g", bufs=3)
                                nc.scalar.activation(
                                    hg[:], psh[:], AF.Gelu,
                                    bias=w1m[:, DC * HID + hc:
                                             DC * HID + hc + 1])
                                h1ch.append(hg)
                            pse = [pe.tile([128, S], f32, tag=f"eo{mc}", bufs=1,
                                           name=f"eo{mc}")
                                   for mc in range(DC)]
                            w2q_last = None
                            for q in range(4):
                                w2q = xp.tile([128, MW2X], f16, tag="w2q",
                                              bufs=3)
                                nc.gpsimd.indirect_dma_start(
                                    out=w2q[:], out_offset=None, in_=mw2_view,
                                    in_offset=bass.IndirectOffsetOnAxis(
                                        ap=off_all[:, j * 5 + 1 + q:
                                                   j * 5 + 2 + q], axis=0))
                                if q == 3:
                                    w2q_last = w2q
                                for hcl in range(3):
                                    hc = 3 * q + hcl
                                    for mc in range(DC):
                                        nc.tensor.matmul(
                                            pse[mc][:],
                                            w2q[:, hcl * D + mc * 128:
                                                hcl * D + (mc + 1) * 128],
                                            h1ch[hc][:],
                                            start=(hc == 0), stop=(hc == HC - 1))
                            # tok += wb * eout; b2 folded in via b2acc (added
                            # with the shared-expert bias below)
                            for mc in range(DC):
                                nc.vector.scalar_tensor_tensor(
                                    tok[s][:, mc, :], pse[mc][:],
                                    wb_all[:, j:j + 1],
                                    tok[s][:, mc, :].bitcast(f32),
                                    op0=ALU.mult, op1=ALU.add)
                            if k == 0:
                                nc.vector.tensor_scalar(
                                    b2acc[s][:], w2q_last[:, 3 * D:3 * D + DC],
                                    wb_all[:, j:j + 1], None, op0=ALU.mult)
                            else:
                                nc.vector.scalar_tensor_tensor(
                                    b2acc[s][:], w2q_last[:, 3 * D:3 * D + DC],
                                    wb_all[:, j:j + 1], b2acc[s][:],
                                    op0=ALU.mult, op1=ALU.add)

                # ---- shared experts ----
                with tc.tile_pool(name="shexp", bufs=1) as sp, \
                     tc.tile_pool(name="ps_s", bufs=1, space="PSUM") as ps:
                    sw1_sb = sp.tile([128, DC, SHID], f32r)
                    sb1_c = sp.tile([128, SC], f32)
                    sb2_c = sp.tile([128, DC], f32)
                    nc.sync.dma_start(sw1_sb[:], _kxm_ap(sw1[i]))
                    nc.sync.dma_start(sb1_c[:], _col_ap(sb1[i]))
                    nc.sync.dma_start(sb2_c[:], _col_ap(sb2[i]))
                    for s in range(BPC):
                        # bias column: shared b2 + weighted routed b2
                        bcomb = sp.tile([128, DC], f32, tag="bcomb", bufs=2)
                        nc.vector.tensor_tensor(bcomb[:], sb2_c[:],
                                                b2acc[s][:], op=ALU.add)
                        pss = [ps.tile([128, S], f32, tag=f"so{mc}", bufs=1,
                                       name=f"so{mc}")
                               for mc in range(DC)]
                        for hc in range(SC):
                            w2c = sp.tile([128, D], f32r, tag="sw2c", bufs=3)
                            nc.sync.dma_start(w2c[:], sw2[i][bass.ts(hc, 128), :])
                            psh = ps.tile([128, S], f32, tag="sh1", bufs=2)
                            for kc in range(DC):
                                nc.tensor.matmul(
                                    psh[:], sw1_sb[:, kc, bass.ts(hc, 128)],
                                    xnm[s][:, kc, :],
                                    start=(kc == 0), stop=(kc == DC - 1))
                            hg = sp.tile([128, S], f32r, tag="sh1g", bufs=3)
                            nc.scalar.activation(hg[:], psh[:], AF.Gelu,
                                                 bias=sb1_c[:, hc:hc + 1])
                            for mc in range(DC):
                                nc.tensor.matmul(
                                    pss[mc][:], w2c[:, bass.ts(mc, 128)], hg[:],
                                    start=(hc == 0), stop=(hc == SC - 1))
                        for mc in range(DC):
                            nc.vector.scalar_tensor_tensor(
                                tok[s][:, mc, :], pss[mc][:],
                                bcomb[:, mc:mc + 1],
                                tok[s][:, mc, :].bitcast(f32),
                                op0=ALU.add, op1=ALU.add)

        # ---------------- final LN + output ----------------
        with tc.tile_pool(name="fin", bufs=1) as pool, \
             tc.tile_pool(name="ps_f", bufs=1, space="PSUM") as pf:
            for s in range(BPC):
                o_sb = pool.tile([128, DC, S], f32, tag="o", bufs=2)
                layernorm(tok[s], o_sb, lnfs_c, lnfb_c, pool, pf)
                nc.sync.dma_start(
                    out_d[s].rearrange("(c p) t -> p c t", p=128), o_sb[:])

    split_multi_waits(nc)
    return nc


# ---------------------------------------------------------------- host wrapper
_CACHE = {}


def _prep_inputs(inputs):
    x = np.asarray(inputs["x"], np.float32)
    # (expert mega-blocks cast to fp16 below)
    # im2col: [B,1,64,2000] -> [B, 256, 500]  (patch pixel index = f*4+dt)
    xT = np.ascontiguousarray(
        x.reshape(B, F, S, 4).transpose(0, 1, 3, 2).reshape(B, PK, S))
    cw = np.ascontiguousarray(
        np.asarray(inputs["conv_w"], np.float32).reshape(D, PK).T)
    posc = np.ascontiguousarray(
        np.asarray(inputs["pos_embed"], np.float32)[0].T
        + np.asarray(inputs["conv_b"], np.float32)[:, None])
    gb2t = np.ascontiguousarray(
        np.broadcast_to(np.asarray(inputs["gate_b2"], np.float32)[:, None, :],
                        (L, BPC, E)))
    # expert mega-blocks: one [128, X] row-block per (layer, expert [,quarter])
    ew1 = np.asarray(inputs["exp_w1"], np.float32)
    eb1 = np.asarray(inputs["exp_b1"], np.float32)
    ew2 = np.asarray(inputs["exp_w2"], np.float32)
    eb2 = np.asarray(inputs["exp_b2"], np.float32)
    mw1 = np.empty((L, E, 128, MW1X), np.float32)
    mw1[..., :DC * HID] = (ew1.reshape(L, E, DC, 128, HID)
                           .transpose(0, 1, 3, 2, 4).reshape(L, E, 128, DC * HID))
    mw1[..., DC * HID:] = (eb1.reshape(L, E, HC, 128).transpose(0, 1, 3, 2))
    mw2 = np.zeros((L, E, 4, 128, MW2X), np.float32)
    mw2[..., :3 * D] = (ew2.reshape(L, E, 4, 3, 128, D)
                        .transpose(0, 1, 2, 4, 3, 5).reshape(L, E, 4, 128, 3 * D))
    mw2[:, :, 3, :, 3 * D:] = (eb2.reshape(L, E, DC, 128)
                               .transpose(0, 1, 3, 2))
    indm = np.zeros((BPC, BPC * 128), np.float32)
    for s in range(BPC):
        indm[s, s * 128:(s + 1) * 128] = 1.0
    iotf = np.arange(128, dtype=np.float32)[:, None]
    common = {
        "cw": cw, "posc": posc,
        "wqkv": np.asarray(inputs["w_qkv"], np.float32),
        "wproj": np.asarray(inputs["w_proj"], np.float32),
        "bproj": np.asarray(inputs["b_proj"], np.float32),
        "ln1s": np.asarray(inputs["ln1_s"], np.float32),
        "ln1b": np.asarray(inputs["ln1_b"], np.float32),
        "ln2s": np.asarray(inputs["ln2_s"], np.float32),
        "ln2b": np.asarray(inputs["ln2_b"], np.float32),
        "gw1": np.asarray(inputs["gate_w1"], np.float32),
        "gb1": np.asarray(inputs["gate_b1"], np.float32),
        "gw2": np.asarray(inputs["gate_w2"], np.float32),
        "gb2t": gb2t,
        "mw1": mw1.astype(np.float16),
        "mw2": mw2.astype(np.float16),
        "indm": indm,
        "iotf": iotf,
        "sw1": np.asarray(inputs["sh_w1"], np.float32),
        "sb1": np.asarray(inputs["sh_b1"], np.float32),
        "sw2": np.asarray(inputs["sh_w2"], np.float32),
        "sb2": np.asarray(inputs["sh_b2"], np.float32),
        "lnfs": np.asarray(inputs["lnf_s"], np.float32),
        "lnfb": np.asarray(inputs["lnf_b"], np.float32),
    }
    in_maps = []
    for c in range(NCORES):
        m = dict(common)
        m["xT"] = xT[c * BPC:(c + 1) * BPC]
        in_maps.append(m)
    return in_maps


def _get_runner():
    """Build (once) a reusable jitted 8-core runner, mirroring
    bass2jax.run_bass_via_pjrt's multi-core path so repeated calls don't
    recompile."""
    if "runner" in _CACHE:
        return _CACHE["runner"]
    import jax
    from jax.sharding import Mesh, PartitionSpec
    from jax.experimental.shard_map import shard_map
    from concourse import bass2jax

    nc = build_nc()
    bass2jax.install_neuronx_cc_hook()

    partition_name = (nc.partition_id_tensor.name
                      if nc.partition_id_tensor else None)
    in_names, out_names, out_avals, zero_shapes = [], [], [], []
    for alloc in nc.m.functions[0].allocations:
        if not isinstance(alloc, mybir.MemoryLocationSet):
            continue
        name = alloc.memorylocations[0].name
        if alloc.kind == "ExternalInput":
            if name != partition_name:
                in_names.append(name)
        elif alloc.kind == "ExternalOutput":
            shape = tuple(alloc.tensor_shape)
            dtype = mybir.dt.np(alloc.dtype)
            out_names.append(name)
            out_avals.append(jax.core.ShapedArray(shape, dtype))
            zero_shapes.append((shape, dtype))
    n_params = len(in_names)
    n_outs = len(out_avals)
    all_in_names = list(in_names) + list(out_names)
    if partition_name is not None:
        all_in_names.append(partition_name)

    def _body(*args):
        operands = list(args)
        if partition_name is not None:
            operands.append(bass2jax.partition_id_tensor())
        outs = bass2jax._bass_exec_p.bind(
            *operands,
            out_avals=tuple(out_avals),
            in_names=tuple(all_in_names),
            out_names=tuple(out_names),
            lowering_input_output_aliases=(),
            sim_require_finite=True,
            sim_require_nnan=True,
            nc=nc,
        )
        return tuple(outs)

    devices = jax.devices()[:NCORES]
    mesh = Mesh(np.asarray(devices), ("core",))
    in_specs = (PartitionSpec("core"),) * (n_params + n_outs)
    out_specs = (PartitionSpec("core"),) * n_outs
    donate = tuple(range(n_params, n_params + n_outs))
    sharded = jax.jit(
        shard_map(_body, mesh=mesh, in_specs=in_specs, out_specs=out_specs,
                  check_rep=False),
        donate_argnums=donate, keep_unused=True)

    def run(in_maps):
        per_core = [[np.asarray(m[name]) for name in in_names] for m in in_maps]
        concat_in = [np.concatenate([per_core[c][i] for c in range(NCORES)],
                                    axis=0) for i in range(n_params)]
        concat_zeros = [np.zeros((NCORES * sh[0], *sh[1:]), dt)
                        for sh, dt in zero_shapes]
        out_arrs = sharded(*concat_in, *concat_zeros)
        out_arrs = [np.asarray(a) for a in out_arrs]
        idx = out_names.index("out")
        return out_arrs[idx].reshape(NCORES, BPC, D, S)

    _CACHE["runner"] = run
    _CACHE["internals"] = {
        "sharded": sharded, "in_names": in_names, "out_names": out_names,
        "zero_shapes": zero_shapes, "mesh": mesh, "n_params": n_params,
    }
    return run


def kernel(**inputs) -> np.ndarray:
    run = _get_runner()
    in_maps = _prep_inputs(inputs)
    out = run(in_maps)                       # [8, 4, 768, 500]
    full = out.reshape(B, D, S).transpose(0, 2, 1)   # [32, 500, 768]
    return np.ascontiguousarray(full)

